# revision 1
# baseline (speedup 1.0000x reference)
"""Trainium2 Bass kernel for nn_CrossAttention (chunked local self-attn + full cross-attn).

Sharding: 8 cores = 2 batches x 4 query-row-blocks (512 rows each), fully SPMD,
no collectives.  Phase 1 (LN1 -> qkv -> chunked local attn (CHUNK=64) -> W_ao ->
+residual -> LN2 -> q_in) is query-row-independent.  Phase 2: each core
projects K/V from its batch's full x (4096 keys) and attends its 512 queries
over all keys.

v2 restructuring vs baseline (TimelineSim 304us -> 301us, but head-loop
packing PE 91%/Act 90%; hardware rel-err 5.4e-3 vs 2e-2 budget):
- All matmul operands bf16 (moving-operand dtype sets PE rate: bf16 = 1
  cycle/row even for small frees, vs plain f32 = 4).  PSUM stays f32.
  Host passes weights / x^T / qx pre-converted to bf16.
- V projection for all 8 heads runs once (v_aug [128,8,65] x32 key-tiles
  SBUF-resident); K^T is projected in 4 groups of 2 heads ([64,4096] per head,
  double-buffered tag rotation), so AV accumulates over all 32 key-tiles in a
  single PSUM bank per head (no inter-chunk accumulate pass).
- V + K0 + K1 projections are emission-interleaved with phase 1 (data-
  independent) at LOW scheduler priority (tc.high_priority(offset=-1e6)), so
  the phase-1 critical chain wins engine arbitration and projections fill
  gaps; K(g+2) interleaves with group g's head loop the same way.
- Softmax exp is the only Activation-engine work in phase 2 (Act is the
  head-loop bottleneck: 16.8M exps/core = ~110us at 0.833ns/col; PSUM->SBUF
  copies are split DVE/Act by phase-window load).
- SBUF is ~188KB/partition usable: dead tiles are tag-aliased (oT_local and
  qinT reuse lnT's space; qin_b reuses ln1's), ao is bf16.
- Hardware-validated rules: matmul operands/outputs at partition base 0 only
  (mixing tile_position row offsets crashes the device); compute-engine
  WRITES at a partition offset pass the BIR verifier only for offsets 0/64
  (DVE at 64 ok, offset 1 rejected; scalar engine rejects any nonzero);
  one matmul output <= one PSUM bank (512 f32); PSUM pools allocate a full
  bank per tag; denominators via ones-column appended to V (row 64 of the
  [65, q] AV output); reciprocal broadcast across partitions with a
  ones-column matmul.
"""

import numpy as np

import concourse.bacc as bacc
import concourse.bass as bass
import concourse.mybir as mybir
import concourse.tile as tile
from concourse.bass_utils import run_bass_kernel_spmd
from concourse.masks import make_identity

F32 = mybir.dt.float32
BF16 = mybir.dt.bfloat16
AF = mybir.ActivationFunctionType
ALU = mybir.AluOpType

H, DH, CHUNK = 8, 64, 64
DIM = 512
INNER = 512
EPS = 1e-5
SCALE = DH ** -0.5

T = 512          # query rows per core
NKT = 4096       # keys (full x length)
NF = DIM // 128  # feature tiles (4)
NT = T // 128    # token tiles per core (4)
NKB = NKT // 512  # 512-key projection blocks (8)
NKTT = NKT // 128  # 128-key tiles (32)
GSZ = 2          # key-tiles per softmax group
NG = NKTT // GSZ  # softmax groups per head (16)
NHG = 4          # K-projection head groups (2 heads each)


def _bcast_ap(dram_ap, parts):
    """[N] DRAM vector -> [parts, N] partition-broadcast AP (for DMA)."""
    return bass.AP(
        tensor=dram_ap.tensor,
        offset=dram_ap.offset,
        ap=[[0, parts]] + [list(x) for x in dram_ap.ap],
    )


def build_nc():
    nc = bacc.Bacc(None, target_bir_lowering=False)

    # ---------------- DRAM I/O ----------------
    qx_d = nc.dram_tensor("qx", [T, DIM], BF16, kind="ExternalInput")
    xT_d = nc.dram_tensor("xT", [DIM, NKT], BF16, kind="ExternalInput")
    Wqkv_d = nc.dram_tensor("Wqkv", [DIM, 3 * INNER], BF16, kind="ExternalInput")
    Wao_d = nc.dram_tensor("Wao", [INNER, DIM], BF16, kind="ExternalInput")
    Wq_d = nc.dram_tensor("Wq", [DIM, INNER], BF16, kind="ExternalInput")
    Wkv_d = nc.dram_tensor("Wkv", [DIM, 2 * INNER], BF16, kind="ExternalInput")
    Wo_d = nc.dram_tensor("Wo", [INNER, DIM], BF16, kind="ExternalInput")
    ln1g_d = nc.dram_tensor("ln1g", [DIM], F32, kind="ExternalInput")
    ln1b_d = nc.dram_tensor("ln1b", [DIM], F32, kind="ExternalInput")
    ln2g_d = nc.dram_tensor("ln2g", [DIM], F32, kind="ExternalInput")
    ln2b_d = nc.dram_tensor("ln2b", [DIM], F32, kind="ExternalInput")
    bao_d = nc.dram_tensor("bao", [DIM], F32, kind="ExternalInput")
    bo_d = nc.dram_tensor("bo", [DIM], F32, kind="ExternalInput")
    qin_d = nc.dram_tensor("qin", [T, DIM], F32, kind="ExternalOutput")
    outT_d = nc.dram_tensor("outT", [DIM, T], F32, kind="ExternalOutput")

    with tile.TileContext(nc) as tc, \
         nc.allow_low_precision(reason="bf16 intermediates; rel-err budget 2e-2"):
        with tc.tile_pool(name="singles", bufs=1) as singles, \
             tc.tile_pool(name="xw", bufs=1) as xw, \
             tc.tile_pool(name="kv", bufs=1) as kv, \
             tc.tile_pool(name="persist", bufs=1) as persist:

            identb = singles.tile([128, 128], BF16)
            make_identity(nc, identb)
            eps_t = singles.tile([128, 1], F32)
            nc.vector.memset(eps_t, EPS)
            ones_t = singles.tile([1, 64], BF16)
            nc.vector.memset(ones_t, 1.0)
            ones8 = singles.tile([128, H, 1], BF16)
            nc.vector.memset(ones8, 1.0)
            ones8h = singles.tile([64, H, 1], BF16)
            nc.vector.memset(ones8h, 1.0)

            g1 = singles.tile([128, DIM], F32)
            b1 = singles.tile([128, DIM], F32)
            g2 = singles.tile([128, DIM], F32)
            b2 = singles.tile([128, DIM], F32)
            bao_bc = singles.tile([128, DIM], F32)
            nc.gpsimd.dma_start(out=g1, in_=_bcast_ap(ln1g_d[:], 128))
            nc.gpsimd.dma_start(out=b1, in_=_bcast_ap(ln1b_d[:], 128))
            nc.gpsimd.dma_start(out=g2, in_=_bcast_ap(ln2g_d[:], 128))
            nc.gpsimd.dma_start(out=b2, in_=_bcast_ap(ln2b_d[:], 128))
            nc.gpsimd.dma_start(out=bao_bc, in_=_bcast_ap(bao_d[:], 128))
            # bo as per-partition columns: bo[m*128 + p] -> bo_col[p, m]
            bo_col = singles.tile([128, NF], F32)
            nc.gpsimd.dma_start(out=bo_col, in_=bo_d[:].rearrange("(m p) -> p m", p=128))

            # ---- input DMAs: qx first (phase-1 dependency), then weights, xT
            qx_t = []
            for tt in range(NT):
                x = persist.tile([128, DIM], BF16, name=f"qx{tt}", tag=f"qx{tt}")
                nc.sync.dma_start(out=x, in_=qx_d[tt * 128:(tt + 1) * 128, :])
                qx_t.append(x)
            Wqkv_sb, Wao_sb, Wq_sb, Wkv_sb, Wo_sb = [], [], [], [], []
            xT_sb = []
            for ft in range(NF):
                w = xw.tile([128, 2 * INNER], BF16, name=f"wkv{ft}", tag=f"wkv{ft}")
                nc.sync.dma_start(out=w, in_=Wkv_d[ft * 128:(ft + 1) * 128, :])
                Wkv_sb.append(w)
            for ft in range(NF):
                xt = xw.tile([128, NKT], BF16, name=f"xT{ft}", tag=f"xT{ft}")
                nc.sync.dma_start(out=xt[:, 0:2048],
                                  in_=xT_d[ft * 128:(ft + 1) * 128, 0:2048])
                xT_sb.append(xt)
            for ft in range(NF):
                w = xw.tile([128, 3 * INNER], BF16, name=f"wqkv{ft}", tag=f"wqkv{ft}")
                nc.sync.dma_start(out=w, in_=Wqkv_d[ft * 128:(ft + 1) * 128, :])
                Wqkv_sb.append(w)
            for ft in range(NF):
                nc.sync.dma_start(out=xT_sb[ft][:, 2048:4096],
                                  in_=xT_d[ft * 128:(ft + 1) * 128, 2048:4096])
                w = xw.tile([128, DIM], BF16, name=f"wao{ft}", tag=f"wao{ft}")
                nc.sync.dma_start(out=w, in_=Wao_d[ft * 128:(ft + 1) * 128, :])
                Wao_sb.append(w)
                w = xw.tile([128, INNER], BF16, name=f"wq{ft}", tag=f"wq{ft}")
                nc.sync.dma_start(out=w, in_=Wq_d[ft * 128:(ft + 1) * 128, :])
                Wq_sb.append(w)
                w = xw.tile([128, DIM], BF16, name=f"wo{ft}", tag=f"wo{ft}")
                nc.sync.dma_start(out=w, in_=Wo_d[ft * 128:(ft + 1) * 128, :])
                Wo_sb.append(w)

            # V for all heads, resident over all 32 key-tiles.
            v_aug = [kv.tile([128, H, 65], BF16, name="v_aug", tag=f"v_aug{kt}")
                     for kt in range(NKTT)]

            # K^T for one 2-head group (double-buffered tag rotation).
            def k_group_tiles(g):
                return [kv.tile([64, NKT], BF16, name=f"kcT{g}_{i}", tag=f"kcT{i}",
                                bufs=2) for i in range(2)]

            def v_proj_block(kb, ps_kv, eng):
                """v_aug for key block kb (4 kt-tiles, all 8 heads).
                Low priority: gap-filler around the critical chain."""
                ctx = tc.high_priority(offset=-1000000)
                ctx.__enter__()
                for kti in range(4):
                    kt = kb * 4 + kti
                    ps = ps_kv.tile([128, 512], F32, name="kv_ps", tag="kv_ps")
                    for ft in range(NF):
                        nc.tensor.matmul(ps[:, :],
                                         xT_sb[ft][:, kt * 128:(kt + 1) * 128],
                                         Wkv_sb[ft][:, INNER:2 * INNER],
                                         start=(ft == 0), stop=(ft == NF - 1))
                    va = v_aug[kt]
                    src = ps[:, :].rearrange("p (h d) -> p h d", h=H)
                    if eng == "v":
                        nc.vector.tensor_copy(va[:, :, 0:64], src)
                    else:
                        nc.scalar.activation(out=va[:, :, 0:64], in_=src,
                                             func=AF.Copy)
                    nc.gpsimd.tensor_copy(va[:, :, 64:65], ones8)
                ctx.__exit__(None, None, None)

            def k_proj_block(g, kb, kcT_g, ps_kv, eng):
                """kcT for heads 2g,2g+1 of key block kb (512 keys).
                Low priority: gap-filler around the critical chain."""
                ctx = tc.high_priority(offset=-1000000)
                ctx.__enter__()
                k0 = kb * 512
                ps = ps_kv.tile([128, 512], F32, name="kv_ps", tag="kv_ps")
                for ft in range(NF):
                    nc.tensor.matmul(ps[:, :],
                                     Wkv_sb[ft][:, g * 128:(g + 1) * 128],
                                     xT_sb[ft][:, k0:k0 + 512],
                                     start=(ft == 0), stop=(ft == NF - 1))
                for half in range(2):
                    dst = kcT_g[half][:, k0:k0 + 512]
                    src = ps[half * 64:(half + 1) * 64, :]
                    if eng == "v":
                        nc.vector.tensor_copy(dst, src)
                    else:
                        nc.scalar.activation(out=dst, in_=src, func=AF.Copy)
                ctx.__exit__(None, None, None)

            def layernorm_tile(pool, x, g_bc, b_bc, prefix, tt, out_dtype):
                stats = pool.tile([128, 6], F32, name="ln_stats", tag="ln_stats")
                nc.vector.bn_stats(out=stats, in_=x)
                mv = pool.tile([128, 2], F32, name="ln_mv", tag="ln_mv")
                nc.vector.bn_aggr(out=mv, in_=stats)
                nc.scalar.activation(out=mv[:, 1:2], in_=mv[:, 1:2], func=AF.Sqrt,
                                     bias=eps_t, scale=1.0)
                nc.vector.reciprocal(out=mv[:, 1:2], in_=mv[:, 1:2])
                y = pool.tile([128, DIM], out_dtype, name=f"{prefix}{tt}",
                              tag=f"{prefix}{tt}", bufs=1)
                nc.vector.tensor_scalar(out=y, in0=x, scalar1=mv[:, 0:1],
                                        scalar2=mv[:, 1:2],
                                        op0=ALU.subtract, op1=ALU.mult)
                nc.vector.tensor_tensor(out=y, in0=y, in1=g_bc, op=ALU.mult)
                nc.vector.tensor_tensor(out=y, in0=y, in1=b_bc, op=ALU.add)
                return y

            qcT = [persist.tile([64, T], BF16, name=f"qcTh{h}", tag=f"qcTh{h}")
                   for h in range(H)]
            oT_norm = [persist.tile([128, T], BF16, name=f"oTn{m}", tag=f"oTn{m}")
                       for m in range(NF)]
            kcT0 = k_group_tiles(0)

            # =========== PHASE 1 (interleaved with V + group-0 K proj) ===========
            with tc.tile_pool(name="p1", bufs=1) as p1, \
                 tc.tile_pool(name="p1w", bufs=4) as p1w, \
                 tc.tile_pool(name="psT", bufs=1, space="PSUM") as psT, \
                 tc.tile_pool(name="psMM", bufs=2, space="PSUM") as psMM, \
                 tc.tile_pool(name="ps1", bufs=1, space="PSUM") as ps1, \
                 tc.tile_pool(name="ps_kv", bufs=2, space="PSUM") as ps_kv:

                # A. LN1 | v_aug blocks 0-1
                ln1 = [layernorm_tile(p1w, qx_t[tt], g1, b1, 'ln1_', tt, BF16)
                       for tt in range(NT)]
                v_proj_block(0, ps_kv, "v")
                v_proj_block(1, ps_kv, "s")

                # B. transpose -> lnT (bf16) | v_aug block 2
                lnT = [p1.tile([128, T], BF16, name=f"lnT{ft}", tag=f"lnT{ft}")
                       for ft in range(NF)]
                for tt in range(NT):
                    for ft in range(NF):
                        tp = psT.tile([128, 128], BF16, name="tposer", tag="tposer")
                        nc.tensor.transpose(
                            tp[:, :], ln1[tt][:, ft * 128:(ft + 1) * 128], identb)
                        nc.vector.tensor_copy(
                            lnT[ft][:, tt * 128:(tt + 1) * 128], tp)
                v_proj_block(2, ps_kv, "v")

                # C. qkv projections | v_aug blocks 3-5
                qT = [p1.tile([64, T], BF16, name=f"qTh{h}", tag=f"qTh{h}")
                      for h in range(H)]
                kT = [p1.tile([64, T], BF16, name=f"kTh{h}", tag=f"kTh{h}")
                      for h in range(H)]
                for m in range(8):  # 4 q tiles + 4 k tiles (transposed outputs)
                    ps = psMM.tile([128, T], F32, name="proj_ps", tag="proj_ps")
                    for ft in range(NF):
                        nc.tensor.matmul(ps[:, :],
                                         Wqkv_sb[ft][:, m * 128:(m + 1) * 128],
                                         lnT[ft][:, :],
                                         start=(ft == 0), stop=(ft == NF - 1))
                    dst = qT if m < 4 else kT
                    mm = m % 4
                    nc.scalar.activation(out=dst[2 * mm], in_=ps[0:64, :],
                                         func=AF.Copy)
                    nc.scalar.activation(out=dst[2 * mm + 1], in_=ps[64:128, :],
                                         func=AF.Copy)
                    if m in (1, 4):
                        v_proj_block(3 + (m > 1), ps_kv, "sv"[m % 2])
                # v_loc with ones column appended per head: [64, H, 65]
                v_loc = [p1.tile([64, H, 65], BF16, name=f"vloc{c}", tag=f"vloc{c}")
                         for c in range(T // CHUNK)]
                for c in range(T // CHUNK):
                    nc.gpsimd.tensor_copy(v_loc[c][:, :, 64:65], ones8h)
                for tt in range(NT):
                    ps = psMM.tile([128, INNER], F32, name="proj_ps", tag="proj_ps")
                    for ft in range(NF):
                        nc.tensor.matmul(ps[:, :],
                                         lnT[ft][:, tt * 128:(tt + 1) * 128],
                                         Wqkv_sb[ft][:, 2 * INNER:3 * INNER],
                                         start=(ft == 0), stop=(ft == NF - 1))
                    for half in range(2):
                        nc.scalar.activation(
                            out=v_loc[2 * tt + half][:, :, 0:64],
                            in_=ps[half * 64:(half + 1) * 64, :].rearrange(
                                "p (h d) -> p h d", h=H),
                            func=AF.Copy)
                v_proj_block(5, ps_kv, "s")

                # D. chunked local attention -> oT_local | v 6-7, K0, K1
                # s^T computed directly (k on partitions): exp writes a^T to
                # SBUF with no transpose; normalization deferred via the
                # ones-column denominators (row 64 of the [65, q] AV output),
                # applied per head after the chunk loop.
                # (oT_local aliases lnT's space: lnT dead after the v_loc proj)
                oT_local = [p1.tile([128, T], BF16, name=f"oTl{m}", tag=f"lnT{m}")
                            for m in range(NF)]
                kcT1 = k_group_tiles(1)
                for cp in range(NT):
                    s_ps = [ps1.tile([64, H, CHUNK], F32, name="s_loc",
                                     tag="s_loc", bufs=2) for c01 in range(2)]
                    for h in range(H):
                        for c01 in range(2):
                            qs = qT[h][:, cp * 128 + c01 * 64: cp * 128 + (c01 + 1) * 64]
                            ks = kT[h][:, cp * 128 + c01 * 64: cp * 128 + (c01 + 1) * 64]
                            nc.tensor.matmul(s_ps[c01][:, h, :], qs, ks,
                                             start=True, stop=True,
                                             tile_position=(0, 0))
                    a_sb = [None, None]
                    for c01 in range(2):
                        a = p1w.tile([64, H, CHUNK], BF16, name="aT_loc",
                                     tag="aT_loc", bufs=2)
                        nc.scalar.activation(out=a, in_=s_ps[c01], func=AF.Exp,
                                             scale=SCALE)
                        sums = p1w.tile([64, H], F32, name="sums_loc",
                                        tag="sums_loc")
                        nc.vector.tensor_reduce(out=sums, in_=a,
                                                axis=mybir.AxisListType.X, op=ALU.add)
                        nc.vector.reciprocal(out=sums, in_=sums)
                        nc.vector.tensor_tensor(out=a, in0=a,
                                                in1=sums.broadcast_to((64, H, CHUNK)),
                                                op=ALU.mult)
                        a_sb[c01] = a
                    for h in range(H):
                        hp, hr = h // 2, (h % 2) * 64
                        av_ps = psT.tile([65, 128], F32, name="av_loc", tag="av_loc")
                        for c01 in range(2):
                            aT_ps = psT.tile([64, 64], BF16, name="tposer",
                                             tag="tposer")
                            nc.tensor.transpose(aT_ps[:, :], a_sb[c01][:, h, :],
                                                identb[0:64, 0:64])
                            aT = p1w.tile([64, 64], BF16, name="aT_sb", tag="aT_sb",
                                          bufs=2)
                            nc.scalar.activation(out=aT, in_=aT_ps, func=AF.Copy)
                            vs = v_loc[cp * 2 + c01][:, h, 0:64]
                            nc.tensor.matmul(av_ps[0:64, c01 * 64:(c01 + 1) * 64],
                                             vs, aT, start=True, stop=True,
                                             tile_position=(0, 0))
                        nc.vector.tensor_copy(
                            oT_local[hp][hr:hr + 64, cp * 128:(cp + 1) * 128],
                            av_ps[0:64, :])
                    if cp < 2:
                        v_proj_block(6 + cp, ps_kv, "sv"[cp % 2])
                        for kb in range(4 * cp, 4 * cp + 4):
                            k_proj_block(0, kb, kcT0, ps_kv, "sv"[kb % 2])
                    else:
                        for kb in range(4 * (cp - 2), 4 * (cp - 1)):
                            k_proj_block(1, kb, kcT1, ps_kv, "sv"[kb % 2])

                # E. W_ao projection + bias + residual -> ao (f32) | K0 blocks 4-7
                ao = [p1.tile([128, DIM], BF16, name=f"ao{tt}", tag=f"ao{tt}")
                      for tt in range(NT)]
                for tt in range(NT):
                    ps = psMM.tile([128, T], F32, name="proj_ps", tag="proj_ps")
                    for ft in range(NF):
                        nc.tensor.matmul(ps[:, :],
                                         oT_local[ft][:, tt * 128:(tt + 1) * 128],
                                         Wao_sb[ft][:, :],
                                         start=(ft == 0), stop=(ft == NF - 1))
                    nc.vector.tensor_tensor(out=ao[tt], in0=ps, in1=bao_bc,
                                            op=ALU.add)
                    nc.vector.tensor_tensor(out=ao[tt], in0=ao[tt], in1=qx_t[tt],
                                            op=ALU.add)

                # F. LN2 -> q_in (f32, DMA out) ; bf16 copy -> transpose -> qinT
                qin = [layernorm_tile(p1w, ao[tt], g2, b2, 'qin_', tt, F32)
                       for tt in range(NT)]
                # (aliases ln1's space: ln1 is dead after the lnT transposes)
                qin_b = [p1w.tile([128, DIM], BF16, name=f"qinb{tt}",
                                  tag=f"ln1_{tt}", bufs=1) for tt in range(NT)]
                for tt in range(NT):
                    nc.sync.dma_start(out=qin_d[tt * 128:(tt + 1) * 128, :],
                                      in_=qin[tt])
                    nc.scalar.activation(out=qin_b[tt], in_=qin[tt], func=AF.Copy)
                # (second rotation of the lnT space: oT_local dead after W_ao)
                qinT = [p1.tile([128, T], BF16, name=f"qinT{ft}", tag=f"lnT{ft}")
                        for ft in range(NF)]
                for tt in range(NT):
                    for ft in range(NF):
                        tp = psT.tile([128, 128], BF16, name="tposer", tag="tposer")
                        nc.tensor.transpose(
                            tp[:, :], qin_b[tt][:, ft * 128:(ft + 1) * 128], identb)
                        nc.scalar.activation(
                            out=qinT[ft][:, tt * 128:(tt + 1) * 128], in_=tp,
                            func=AF.Copy)

                # G. W_q projection -> qcT (persists)
                for m in range(NF):
                    ps = psMM.tile([128, T], F32, name="proj_ps", tag="proj_ps")
                    for ft in range(NF):
                        nc.tensor.matmul(ps[:, :],
                                         Wq_sb[ft][:, m * 128:(m + 1) * 128],
                                         qinT[ft][:, :],
                                         start=(ft == 0), stop=(ft == NF - 1))
                    nc.scalar.activation(out=qcT[2 * m], in_=ps[0:64, :],
                                         func=AF.Copy)
                    nc.scalar.activation(out=qcT[2 * m + 1], in_=ps[64:128, :],
                                         func=AF.Copy)

            # =========== PHASE 2: cross-attention head loops ===========
            # 4 groups of 2 heads; group g+1's K proj interleaves with group
            # g's head loop.
            with tc.tile_pool(name="pa", bufs=4) as pa_pool, \
                 tc.tile_pool(name="fin", bufs=2) as fin, \
                 tc.tile_pool(name="ps_s", bufs=2, space="PSUM") as ps_s, \
                 tc.tile_pool(name="ps_o", bufs=2, space="PSUM") as ps_o, \
                 tc.tile_pool(name="ps_kv2", bufs=1, space="PSUM") as ps_kv2, \
                 tc.tile_pool(name="ps_bc", bufs=1, space="PSUM") as ps_bc:

                kcT_all = [kcT0, kcT1, None, None]
                for g in range(NHG):
                    kcT_g = kcT_all[g]
                    for hi in range(2):
                        h = 2 * g + hi
                        o_ps = ps_o.tile([65, T], F32, name="o_ps", tag="o_ps")
                        for grp in range(NG):
                            s_ps = ps_s.tile([128, GSZ, T], F32, name="s_ps",
                                             tag="s_ps")
                            for j in range(GSZ):
                                kt = grp * GSZ + j
                                nc.tensor.matmul(
                                    s_ps[:, j, :],
                                    kcT_g[hi][:, kt * 128:(kt + 1) * 128],
                                    qcT[h][:, :],
                                    start=True, stop=True,
                                    tile_position=(0, 0))
                            a_sb = pa_pool.tile([128, GSZ, T], BF16, name="a_sb",
                                                tag="a_sb")
                            nc.scalar.activation(out=a_sb, in_=s_ps, func=AF.Exp,
                                                 scale=SCALE)
                            for j in range(GSZ):
                                kt = grp * GSZ + j
                                nc.tensor.matmul(
                                    o_ps[:, :],
                                    v_aug[kt][:, h, :],
                                    a_sb[:, j, :],
                                    start=(grp == 0 and j == 0),
                                    stop=(grp == NG - 1 and j == GSZ - 1))
                        # normalize: rec broadcast via ones-column matmul
                        rec = fin.tile([1, T], BF16, name="rec", tag="rec")
                        nc.vector.reciprocal(rec, o_ps[64:65, :])
                        bc_ps = ps_bc.tile([64, T], F32, name="bc_ps", tag="bc_ps")
                        nc.tensor.matmul(bc_ps[:, :], ones_t[0:1, :], rec[0:1, :],
                                         start=True, stop=True)
                        o_sb = fin.tile([64, T], F32, name="o_sb", tag="o_sb")
                        nc.vector.tensor_copy(o_sb, o_ps[0:64, :])
                        nc.vector.tensor_tensor(out=oT_norm[g][hi * 64:hi * 64 + 64, :],
                                                in0=o_sb, in1=bc_ps, op=ALU.mult)
                        # interleave K projection for group g+2 (tag rotation
                        # aliases group g's space; WAR waits until g's reads done)
                        if g < NHG - 2:
                            if hi == 0:
                                kcT_all[g + 2] = k_group_tiles(g + 2)
                            for kb in range(4 * hi, 4 * hi + 4):
                                k_proj_block(g + 2, kb, kcT_all[g + 2], ps_kv2, "v")

            # =========== W_o + bias -> outT ===========
            with tc.tile_pool(name="fin2", bufs=2) as fin2, \
                 tc.tile_pool(name="ps_f", bufs=2, space="PSUM") as ps_f:
                for m in range(NF):
                    ps = ps_f.tile([128, T], F32, name="out_ps", tag="out_ps")
                    for ft in range(NF):
                        nc.tensor.matmul(ps[:, :],
                                         Wo_sb[ft][:, m * 128:(m + 1) * 128],
                                         oT_norm[ft][:, :],
                                         start=(ft == 0), stop=(ft == NF - 1))
                    ot = fin2.tile([128, T], F32, name="outT_sb", tag="outT_sb",
                                   bufs=2)
                    nc.vector.tensor_scalar(out=ot, in0=ps,
                                            scalar1=bo_col[:, m:m + 1],
                                            scalar2=None, op0=ALU.add)
                    nc.sync.dma_start(out=outT_d[m * 128:(m + 1) * 128, :],
                                      in_=ot)

    nc.finalize()
    return nc


_NC_CACHE = {}


def kernel(x, q_x, ln1_g, ln1_b, W_qkv, W_ao, b_ao, ln2_g, ln2_b,
           W_q, W_kv, W_o, b_o):
    import ml_dtypes
    BF = ml_dtypes.bfloat16
    B, NQ, _ = q_x.shape
    n_blocks = 4
    rows = NQ // n_blocks

    if "nc" not in _NC_CACHE:
        _NC_CACHE["nc"] = build_nc()
    nc = _NC_CACHE["nc"]

    xT = np.ascontiguousarray(
        np.asarray(x, np.float32).transpose(0, 2, 1)).astype(BF)
    qxb = np.asarray(q_x, np.float32).astype(BF)
    common = {
        "Wqkv": np.ascontiguousarray(np.asarray(W_qkv, np.float32).astype(BF)),
        "Wao": np.ascontiguousarray(np.asarray(W_ao, np.float32).astype(BF)),
        "Wq": np.ascontiguousarray(np.asarray(W_q, np.float32).astype(BF)),
        "Wkv": np.ascontiguousarray(np.asarray(W_kv, np.float32).astype(BF)),
        "Wo": np.ascontiguousarray(np.asarray(W_o, np.float32).astype(BF)),
        "ln1g": np.ascontiguousarray(ln1_g, np.float32),
        "ln1b": np.ascontiguousarray(ln1_b, np.float32),
        "ln2g": np.ascontiguousarray(ln2_g, np.float32),
        "ln2b": np.ascontiguousarray(ln2_b, np.float32),
        "bao": np.ascontiguousarray(b_ao, np.float32),
        "bo": np.ascontiguousarray(b_o, np.float32),
    }
    in_maps = []
    for c in range(8):
        b, r = c // n_blocks, c % n_blocks
        m = dict(common)
        m["qx"] = np.ascontiguousarray(qxb[b, r * rows:(r + 1) * rows, :])
        m["xT"] = xT[b]
        in_maps.append(m)

    res = run_bass_kernel_spmd(nc, in_maps, core_ids=list(range(8)))

    out = np.empty((B, NQ, DIM), np.float32)
    q_in = np.empty((B, NQ, DIM), np.float32)
    for c in range(8):
        b, r = c // n_blocks, c % n_blocks
        q_in[b, r * rows:(r + 1) * rows, :] = res.results[c]["qin"]
        out[b, r * rows:(r + 1) * rows, :] = res.results[c]["outT"].T
    return (out, q_in)

